# revision 22
# baseline (speedup 1.0000x reference)
"""Multi-head self-attention on 8 TRN2 NeuronCores.

Problem: x[2,2048,1024] -> qkv proj -> 16-head attention -> out proj.
Sharding: core c handles batch b=c//4 and head group g=c%4 (4 heads each).
Each core computes a partial output y_c[2048,1024] = attn_out_heads(g) @ W_proj[rows g];
host sums the 4 partials per batch and adds b_proj.

Design (v2, bf16):
  - All matmul operands bf16 (host pre-converts); PSUM accumulates fp32.
  - k stored per head-pair: kp[pair][128, S] with even head's 64 dims in
    partitions 0-63, odd head's in 64-127 (no zero padding). Score matmuls
    run K=64 row-tiled: the even-head MM at tile_position (0,0) and the
    odd-head MM at (64,0) execute CONCURRENTLY on disjoint PE row groups,
    halving score time vs the padded-K=128 formulation.
  - q likewise: qk[pair][128, S] (q_even rows 0-63, q_odd 64-127).
  - exp() on ScalarE straight out of PSUM in [128,1024] tiles, 1/sqrt(hd)
    scale fused, bf16 out; no max-subtraction (scores bounded).
  - PV: out^T[d,i] = v_aug.T @ exp^T per head with a ones column appended
    (v_aug [128j, 65]) so the matmul also yields softmax denominators.
  - Pipeline: 4 combos (pair x i-half). The ACT exp chain (~133us) paces the
    kernel; PV of the previous combo, QKV blocks and out-proj blocks are
    dripped into PE slack from a cost-paced FIFO. The last combo's even-head
    PV is front-loaded inline (2 spare PSUM banks) to shrink the tail.
"""

import numpy as np

N_CORES = 8
B, S, D = 2, 2048, 1024
H, HD = 16, 64
HPC = 4          # heads per core
F_QK = 512       # q+k features per core (4 heads x 64 x 2)
F_V = 256        # v features per core
FT = 768         # total qkv features per core
SC = 512         # seq chunk (matmul N)
NSC = S // SC    # 4
NJ = S // 128    # 16 j-blocks
NDC = D // 128   # 8 contraction chunks

_CACHE = {}
DEBUG_DUMP = False


def _build(repeat=1):
    import contextlib
    import concourse.bass as bass  # noqa: F401
    import concourse.mybir as mybir
    import concourse.tile as tile
    from concourse import bacc

    F32, BF16 = mybir.dt.float32, mybir.dt.bfloat16
    AF = mybir.ActivationFunctionType

    nc = bacc.Bacc("TRN2", target_bir_lowering=False, num_devices=N_CORES)
    xT = nc.declare_dram_parameter("xT", [D, S], BF16, isOutput=False)
    W1 = nc.declare_dram_parameter("W1", [D, FT], BF16, isOutput=False)
    b1 = nc.declare_dram_parameter("b1", [FT, 1], F32, isOutput=False)
    Wp = nc.declare_dram_parameter("Wp", [HPC * HD, D], BF16, isOutput=False)
    y = nc.declare_dram_parameter("y", [S, D], F32, isOutput=True)
    nc_dbg = {}
    if DEBUG_DUMP:
        for nm, shp in (("dbg_qk0", [128, S]), ("dbg_kp0", [128, S]),
                        ("dbg_v40", [128, HPC * (HD + 2)]),
                        ("dbg_ex0", [128, 1024]), ("dbg_outT0", [128, S])):
            nc_dbg[nm] = nc.declare_dram_parameter(nm, shp, BF16, isOutput=True)
        F32_ = F32
        nc_dbg["dbg_posb"] = nc.declare_dram_parameter(
            "dbg_posb", [HD + 1, SC], F32_, isOutput=True)
        nc_dbg["dbg_rb"] = nc.declare_dram_parameter(
            "dbg_rb", [HD, SC], F32_, isOutput=True)

    with tile.TileContext(nc) as tc:
        with (
            tc.tile_pool(name="weights", bufs=1) as wpool,
            tc.tile_pool(name="persist", bufs=1) as persist,
            tc.tile_pool(name="xin", bufs=8) as xpool,
            tc.tile_pool(name="etile", bufs=22) as epool,
            tc.tile_pool(name="yout", bufs=3) as ypool,
            tc.tile_pool(name="small", bufs=3) as spool,
            tc.tile_pool(name="psSS", bufs=2, space="PSUM") as psSS,
            tc.tile_pool(name="psMM", bufs=2, space="PSUM") as psMM,
            tc.tile_pool(name="psP3", bufs=2, space="PSUM") as psP3,
        ):
            # ---- load weights / biases (outside the repeat loop) ----
            w1t = []
            for dc in range(NDC):
                t = wpool.tile([128, FT], BF16, tag=f"w1_{dc}")
                nc.sync.dma_start(out=t, in_=W1[dc * 128:(dc + 1) * 128, :])
                w1t.append(t)
            wpt = []
            for p in range(2):
                t = wpool.tile([128, D], BF16, tag=f"wp_{p}", name=f"wp_{p}")
                nc.sync.dma_start(out=t, in_=Wp[p * 128:(p + 1) * 128, :])
                wpt.append(t)
            bqk = []
            for fb in range(4):
                t = wpool.tile([128, 1], F32, tag=f"bqk_{fb}")
                nc.sync.dma_start(out=t, in_=b1[fb * 128:(fb + 1) * 128, :])
                bqk.append(t)
            bv = wpool.tile([128, F_V], F32, tag="bv")
            bvsrc = b1[F_QK:FT, 0:1]
            bv_ap = bass.AP(tensor=bvsrc.tensor, offset=bvsrc.offset,
                            ap=[[0, 128], [1, F_V]])
            nc.sync.dma_start(out=bv, in_=bv_ap)

            # persistent activation tiles (written each iter, ones cols once)
            qk = [persist.tile([128, S], BF16, tag=f"qk_{p}", name=f"qk_{p}")
                  for p in range(2)]
            kp = [persist.tile([128, S], BF16, tag=f"kp_{p}", name=f"kp_{p}")
                  for p in range(2)]
            v4 = [persist.tile([128, HPC * (HD + 2)], BF16, tag=f"v4_{jc}",
                               name=f"v4_{jc}") for jc in range(NJ)]
            outT = [persist.tile([128, S], BF16, tag=f"outT_{p}",
                                 name=f"outT_{p}") for p in range(2)]
            for jc in range(NJ):
                for h in range(HPC):
                    nc.vector.memset(
                        v4[jc][:, h * (HD + 2) + HD:h * (HD + 2) + HD + 1], 1.0)
            # prologue activation so the exp table set is resident pre-loop
            dum = wpool.tile([128, 8], BF16, tag="dummy")
            dum2 = wpool.tile([128, 8], BF16, tag="dummy2")
            nc.vector.memset(dum, 0.0)
            nc.scalar.activation(dum2, dum, AF.Exp, bias=0.0, scale=1.0)

            if repeat > 1:
                ET = mybir.EngineType
                loop_cm = tc.For_i(0, repeat, 1,
                                   hint_engines=(ET.PE, ET.DVE, ET.Activation,
                                                 ET.Pool, ET.SP))
            else:
                loop_cm = contextlib.nullcontext()
            with loop_cm:
                _emit_body(nc, tc, mybir, locals())
    nc.compile()
    return nc


def _emit_body(nc, tc, mybir, env):
    F32, BF16 = mybir.dt.float32, mybir.dt.bfloat16
    AF = mybir.ActivationFunctionType
    w1t, wpt, bqk, bv = (env[k] for k in ("w1t", "wpt", "bqk", "bv"))
    qk, kp, v4, outT = (env[k] for k in ("qk", "kp", "v4", "outT"))
    xT, y = env["xT"], env["y"]
    xpool, epool, ypool, spool = (
        env[k] for k in ("xpool", "epool", "ypool", "spool"))
    psSS, psMM, psP3 = env["psSS"], env["psMM"], env["psP3"]

    # ---- x input tiles (one [128, S] DMA per contraction chunk) ----
    xtf = []
    for dc in range(NDC):
        t = xpool.tile([128, S], BF16, tag="xt", name=f"xt_{dc}")
        for sc in range(NSC):
            nc.sync.dma_start(out=t[:, sc * SC:(sc + 1) * SC],
                              in_=xT[dc * 128:(dc + 1) * 128,
                                     sc * SC:(sc + 1) * SC])
        xtf.append(t)

    # ---- step builders (lists of (pe_cost_ns, closure)) ----
    def qkv_block(fb, sc):
        # q/k projection block: pq[128f, 512s] = W1[:,fb].T @ x ; evac + bias
        st = {}
        ssl = slice(sc * SC, (sc + 1) * SC)
        dst = qk[fb] if fb < 2 else kp[fb - 2]

        def mm(dc):
            if dc == 0:
                st["pq"] = psMM.tile([128, SC], F32, tag="mm", name="pq")
            nc.tensor.matmul(st["pq"], w1t[dc][:, fb * 128:(fb + 1) * 128],
                             xtf[dc][:, sc * SC:(sc + 1) * SC],
                             start=(dc == 0), stop=(dc == NDC - 1))

        def evac():
            nc.vector.tensor_scalar_add(dst[:, ssl], st["pq"], bqk[fb])
        return [(240, lambda dc=dc: mm(dc)) for dc in range(NDC)] + [(0, evac)]

    def v_block(sc, sb):
        # v projection for s-block jc: pv[128s, 256f] = x_sb.T @ Wv ; -> v4
        st = {}
        jc = sc * 4 + sb

        def mm(dc):
            if dc == 0:
                st["pv"] = psMM.tile([128, F_V], F32, tag="mm", name="pv")
            c0 = sc * SC + sb * 128
            nc.tensor.matmul(st["pv"], xtf[dc][:, c0:c0 + 128],
                             w1t[dc][:, F_QK:FT], start=(dc == 0),
                             stop=(dc == NDC - 1))

        def evac():
            for h in range(HPC):
                nc.vector.tensor_add(v4[jc][:, h * (HD + 2):h * (HD + 2) + HD],
                                     st["pv"][:, h * HD:(h + 1) * HD],
                                     bv[:, h * HD:(h + 1) * HD])
        return [(135, lambda dc=dc: mm(dc)) for dc in range(NDC)] + [(0, evac)]

    def norm_steps(po_get, p, lr, ic, half):
        # normalize po[65,512] by its denominator row -> outT
        def go():
            posb = spool.tile([HD + 1, SC], F32, tag="posb")
            nc.vector.tensor_copy(posb, po_get())
            den0 = spool.tile([1, SC], F32, tag="den0")
            nc.vector.tensor_copy(den0, posb[HD:HD + 1, :])
            recip = spool.tile([1, SC], F32, tag="recip")
            nc.vector.reciprocal_approx_fast(recip, den0)
            rb = spool.tile([HD, SC], F32, tag="rb")
            nc.gpsimd.partition_broadcast(rb, recip)
            import kernel as _K
            if _K.DEBUG_DUMP and (p, lr, ic, half) == (0, 0, 0, 0):
                d = env["nc_dbg"]
                nc.sync.dma_start(out=d["dbg_posb"][:, :], in_=posb)
                nc.sync.dma_start(out=d["dbg_rb"][:, :], in_=rb)
            isl = slice(ic * 1024 + half * SC, ic * 1024 + (half + 1) * SC)
            if lr == 0:
                nc.vector.tensor_mul(outT[p][0:HD, isl], posb[0:HD, :], rb)
            else:
                tmp = spool.tile([HD, SC], BF16, tag="tmp64")
                nc.vector.tensor_mul(tmp, posb[0:HD, :], rb)
                nc.sync.dma_start(out=outT[p][HD:128, isl], in_=tmp)
        return [(0, go)]

    def pv_pass(p, lr, ic, half, exl):
        # po[65,512] = v_aug.T @ exp over all 16 j-blocks (accumulating)
        st = {}
        h = 2 * p + lr
        hsl = slice(h * (HD + 2), h * (HD + 2) + HD + 1)
        esl = slice(half * SC, (half + 1) * SC)

        def mm(jc):
            if jc == 0:
                st["po"] = psMM.tile([HD + 1, SC], F32, tag="mm", name="po")
            nc.tensor.matmul(st["po"], v4[jc][:, hsl], exl[jc][:, esl],
                             start=(jc == 0), stop=(jc == NJ - 1))
        return ([(240, lambda jc=jc: mm(jc)) for jc in range(NJ)]
                + norm_steps(lambda: st["po"], p, lr, ic, half))

    def proj_block(sblk, oc):
        st = {}
        ssl = slice(sblk * 128, (sblk + 1) * 128)
        osl = slice(oc * SC, (oc + 1) * SC)

        def mm(pp):
            if pp == 0:
                st["py"] = psMM.tile([128, SC], F32, tag="mm", name="py")
            nc.tensor.matmul(st["py"], outT[pp][:, ssl], wpt[pp][:, osl],
                             start=(pp == 0), stop=(pp == 1))

        def evac():
            ysb = ypool.tile([128, SC], F32, tag="ysb", name="ysb")
            nc.vector.tensor_copy(ysb, st["py"])
            nc.sync.dma_start(out=y[ssl, osl], in_=ysb)
        return [(240, lambda: mm(0)), (240, lambda: mm(1)), (0, evac)]

    fifo = []

    def pump(budget):
        spent = 0
        while fifo and spent < budget:
            cost, fn = fifo.pop(0)
            fn()
            spent += cost

    def drain():
        while fifo:
            fifo.pop(0)[1]()

    def run(steps):
        for _, fn in steps:
            fn()

    def _emit_po3(jc, po3, exe):
        if jc < 0:
            return
        hsl = slice(2 * (HD + 2), 2 * (HD + 2) + HD + 1)   # head 2 of core
        for half in (0, 1):
            if jc == 0:
                po3[half] = psP3.tile([HD + 1, SC], F32, tag="po3",
                                      name=f"po3_{half}")
            nc.tensor.matmul(po3[half], v4[jc][:, hsl],
                             exe[jc][:, half * SC:(half + 1) * SC],
                             start=(jc == 0), stop=(jc == NJ - 1))

    # ---- upfront: minimum to start combo 0's score stream ----
    run(qkv_block(2, 0))       # k pair0, s-chunk 0 (j-blocks 0-3)
    run(qkv_block(0, 0))       # q pair0, s-chunk 0 (i cols 0-511)
    run(qkv_block(0, 1))       # q pair0, s-chunk 1 (i cols 512-1023)

    # ---- combo loop: (pair, i-half) ----
    EXS = {}
    for ci, (p, ic) in enumerate(((0, 0), (1, 0), (0, 1), (1, 1))):
        if ci == 0:
            for sc in (1, 2, 3):
                fifo.extend(qkv_block(2, sc))      # rest of k pair0
            for sc in range(NSC):
                fifo.extend(qkv_block(3, sc))      # k pair1
            fifo.extend(qkv_block(1, 0))           # q pair1 i-half 0
            fifo.extend(qkv_block(1, 1))
            for sc in range(NSC):
                for sb in range(4):
                    fifo.extend(v_block(sc, sb))
        elif ci == 1:
            for lr in (0, 1):
                for half in (0, 1):
                    fifo.extend(pv_pass(0, lr, 0, half, EXS[(0, 0, lr)]))
            fifo.extend(qkv_block(0, 2))           # q pair0 i-half 1
            fifo.extend(qkv_block(0, 3))
        elif ci == 2:
            for lr in (0, 1):
                for half in (0, 1):
                    fifo.extend(pv_pass(1, lr, 0, half, EXS[(1, 0, lr)]))
            fifo.extend(qkv_block(1, 2))           # q pair1 i-half 1
            fifo.extend(qkv_block(1, 3))
        else:
            for lr in (0, 1):
                for half in (0, 1):
                    fifo.extend(pv_pass(0, lr, 1, half, EXS[(0, 1, lr)]))
            for sblk in range(8):
                for oc in range(2):
                    fifo.extend(proj_block(sblk, oc))

        budget = (sum(c for c, _ in fifo) + NJ - 2) // (NJ - 1) + 60

        exe = [None] * NJ
        exo = [None] * NJ
        EXS[(p, ic, 0)] = exe
        EXS[(p, ic, 1)] = exo
        po3 = {}
        for jc in range(NJ):
            jsl = slice(jc * 128, (jc + 1) * 128)
            ss_e = psSS.tile([128, 1024], F32, tag="ss", name=f"ss{ci}_{jc}e")
            ss_o = psSS.tile([128, 1024], F32, tag="ss", name=f"ss{ci}_{jc}o")
            for half in (0, 1):
                isl = slice(ic * 1024 + half * SC, ic * 1024 + (half + 1) * SC)
                osl = slice(half * SC, (half + 1) * SC)
                nc.tensor.matmul(ss_e[:, osl], kp[p][0:HD, jsl],
                                 qk[p][0:HD, isl], start=True, stop=True)
                nc.tensor.matmul(ss_o[:, osl], kp[p][HD:128, jsl],
                                 qk[p][HD:128, isl], start=True, stop=True)
            exe[jc] = epool.tile([128, 1024], BF16, tag="ex_e",
                                 name=f"exe{ci}_{jc}")
            nc.scalar.activation(exe[jc], ss_e, AF.Exp, bias=0.0, scale=0.125)
            exo[jc] = epool.tile([128, 1024], BF16, tag="ex_o",
                                 name=f"exo{ci}_{jc}")
            nc.scalar.activation(exo[jc], ss_o, AF.Exp, bias=0.0, scale=0.125)
            if DEBUG_DUMP and ci == 0 and jc == 0:
                nc.sync.dma_start(out=env["nc_dbg"]["dbg_ex0"][:, :],
                                  in_=exe[jc])
            if ci == 3:
                # front-load last combo's even-head PV into spare PSUM banks
                # (one-jc lag so the matmuls never wait on the live exp)
                _emit_po3(jc - 1, po3, exe)
            pump(budget)
        if ci == 3:
            _emit_po3(NJ - 1, po3, exe)
        drain()
        if ci == 3:
            for half in (0, 1):
                run(norm_steps(lambda half=half: po3[half], 1, 0, 1, half))

    # ---- tail: odd head of (pair1, i-half1) + its projection ----
    for half in (0, 1):
        fifo.extend(pv_pass(1, 1, 1, half, EXS[(1, 1, 1)]))
    for sblk in range(8, 16):
        for oc in range(2):
            fifo.extend(proj_block(sblk, oc))
    drain()
    if DEBUG_DUMP:
        d = env["nc_dbg"]
        nc.sync.dma_start(out=d["dbg_qk0"][:, :], in_=qk[0])
        nc.sync.dma_start(out=d["dbg_kp0"][:, :], in_=kp[0])
        nc.sync.dma_start(out=d["dbg_v40"][:, :], in_=v4[0])
        nc.sync.dma_start(out=d["dbg_outT0"][:, :], in_=outT[0])


def _shards(x, W_qkv, b_qkv, W_proj):
    """Build per-core input maps (matmul operands pre-converted to bf16)."""
    import ml_dtypes
    bf16 = ml_dtypes.bfloat16
    xTb = [np.ascontiguousarray(x[b].T.astype(bf16)) for b in range(B)]
    in_maps = []
    for c in range(N_CORES):
        b, g = c // 4, c % 4
        cols = slice(g * HPC * HD, (g + 1) * HPC * HD)  # 256 cols within q/k/v
        W1 = np.concatenate([W_qkv[:, 0 * D:1 * D][:, cols],
                             W_qkv[:, 1 * D:2 * D][:, cols],
                             W_qkv[:, 2 * D:3 * D][:, cols]], axis=1)
        b1 = np.concatenate([b_qkv[0 * D:1 * D][cols],
                             b_qkv[1 * D:2 * D][cols],
                             b_qkv[2 * D:3 * D][cols]]).reshape(FT, 1)
        Wp = W_proj[g * HPC * HD:(g + 1) * HPC * HD, :]
        in_maps.append({
            "xT": xTb[b],
            "W1": np.ascontiguousarray(W1.astype(bf16)),
            "b1": np.ascontiguousarray(b1, dtype=np.float32),
            "Wp": np.ascontiguousarray(Wp.astype(bf16)),
        })
    return in_maps


def kernel(x, W_qkv, b_qkv, W_proj, b_proj):
    from concourse.bass_utils import run_bass_kernel_spmd

    x = np.asarray(x, dtype=np.float32)
    W_qkv = np.asarray(W_qkv, dtype=np.float32)
    b_qkv = np.asarray(b_qkv, dtype=np.float32)
    W_proj = np.asarray(W_proj, dtype=np.float32)
    b_proj = np.asarray(b_proj, dtype=np.float32)

    if "nc" not in _CACHE:
        _CACHE["nc"] = _build()
    nc = _CACHE["nc"]

    in_maps = _shards(x, W_qkv, b_qkv, W_proj)
    res = run_bass_kernel_spmd(nc, in_maps, list(range(N_CORES)), trace=False)

    out = np.empty((B, S, D), dtype=np.float32)
    for b in range(B):
        acc = res.results[4 * b]["y"].astype(np.float32)
        for g in range(1, 4):
            acc = acc + res.results[4 * b + g]["y"]
        out[b] = acc + b_proj[None, :]
    return out


if __name__ == "__main__":
    rng = np.random.default_rng(0)
    scale = 1.0 / np.sqrt(D)
    inputs = {
        "x": rng.standard_normal((B, S, D), dtype=np.float32),
        "W_qkv": (rng.standard_normal((D, 3 * D)).astype(np.float32) * scale),
        "b_qkv": np.zeros(3 * D, np.float32),
        "W_proj": (rng.standard_normal((D, D)).astype(np.float32) * scale),
        "b_proj": np.zeros(D, np.float32),
    }
    out = kernel(**inputs)
    print("out", out.shape, out.dtype, np.abs(out).max())
